# revision 1
# baseline (speedup 1.0000x reference)
"""GATv2 (3-layer) + attentive pooling + MLP head.

Self-contained: accepts FULL unsharded inputs, returns FULL [B, 1] output.

Implementation note: the Neuron compiler on this platform rejects the
sort-based scatter lowering XLA emits for data-dependent segment_sum /
segment_max ([NCC_EVRF029] "Operation sort is not supported on trn2"),
so the graph portion cannot be lowered through PJRT here. The model is
computed with NumPy using a single host-side stable sort of edges by
destination plus np.add.reduceat / np.maximum.reduceat segment
reductions; every node has a self-loop, so all destination segments are
non-empty and reduceat is exact.
"""
import numpy as np

N = 20000
E = 200000
B = 512
H = 8
C = 64
NEG_SLOPE = np.float32(0.2)


def _layer(x, Wl, Wr, att, b, src_s, dst_s, starts, concat):
    n = x.shape[0]
    xl = (x @ Wl).reshape(n, H, C)
    xr = (x @ Wr).reshape(n, H, C)
    e = xl[src_s] + xr[dst_s]
    e = np.where(e > 0, e, NEG_SLOPE * e)
    logits = np.einsum('ehc,hc->eh', e, att, dtype=np.float32)
    m = np.maximum.reduceat(logits, starts, axis=0)
    ex = np.exp(logits - m[dst_s])
    s = np.add.reduceat(ex, starts, axis=0)
    alpha = ex / (s[dst_s] + np.float32(1e-16))
    out = np.add.reduceat(xl[src_s] * alpha[:, :, None], starts, axis=0)
    out = out.reshape(n, H * C) if concat else out.mean(axis=1, dtype=np.float32)
    return (out + b).astype(np.float32)


def kernel(**inputs):
    f32 = lambda k: np.asarray(inputs[k], np.float32)
    x = f32("x")
    ei = np.asarray(inputs["edge_index"], np.int64)
    batch_index = np.asarray(inputs["batch_index"], np.int64)

    loop = np.arange(N, dtype=np.int64)
    src = np.concatenate([ei[0], loop])
    dst = np.concatenate([ei[1], loop])
    order = np.argsort(dst, kind="stable")
    src_s = src[order]
    dst_s = dst[order]
    # self-loops guarantee every node has >=1 incoming edge
    starts = np.searchsorted(dst_s, np.arange(N))

    h = _layer(x, f32("Wl0"), f32("Wr0"), f32("att0"), f32("b0"),
               src_s, dst_s, starts, True)
    h = _layer(h, f32("Wl1"), f32("Wr1"), f32("att1"), f32("b1"),
               src_s, dst_s, starts, True)
    h = _layer(h, f32("Wl2"), f32("Wr2"), f32("att2"), f32("b2"),
               src_s, dst_s, starts, False)

    w = 1.0 / (1.0 + np.exp(-(h @ f32("w_aw") + f32("b_aw"))))
    w = w.astype(np.float32)

    counts = np.bincount(batch_index, minlength=B)
    bstarts = np.minimum(np.searchsorted(batch_index, np.arange(B)), N - 1)
    p_max = np.maximum.reduceat(h, bstarts, axis=0)
    p_sum = np.add.reduceat(w * h, bstarts, axis=0)
    empty = counts == 0
    p_max[empty] = 0.0
    p_sum[empty] = 0.0

    g = np.concatenate([p_max, p_sum], axis=1).astype(np.float32)
    z = g @ f32("Wm1") + f32("bm1")
    a = f32("a_prelu")
    z = np.where(z > 0, z, a * z).astype(np.float32)
    return (z @ f32("Wm2") + f32("bm2")).astype(np.float32)



# revision 10
# speedup vs baseline: 14548.6981x; 14548.6981x over previous
"""GATv2 (3 layers, 8 heads) + attentive pooling + MLP head on 8 TRN2 NeuronCores.

Strategy (sharding_hint): nodes and their incoming edges are partitioned by
destination node across the 8 cores (2500 dst nodes each). Per layer, each
core computes its shard of xl = h@Wl / xr = h@Wr on the TensorEngine,
all-gathers xl so any core can fetch arbitrary source rows, then runs the
edge pipeline: dma_gather of xl[src] / xr[dst] rows, e = xl+xr (DVE),
leaky-relu (one fused DVE scalar_tensor_tensor), per-head dot with att
(DVE mul + windowed reduce), exp (ScalarE), and segment-softmax-weighted
sums via TensorEngine matmuls against host-built 0/1 destination one-hot
matrices accumulated in PSUM. Graph pooling: weighted-sum pooling via a
node->graph one-hot matmul, max pooling via a masked running-max scan +
ap_gather, both all-reduced across cores; the tiny MLP head runs
redundantly on every core.

The whole feature path is fp32: the harness gate is max relative error on
outputs with tiny elements, so bf16 feature quantization does not pass.

Self-contained: takes FULL inputs, returns FULL [512, 1] output.
"""

import numpy as np

N = 20000
E = 200000
B = 512
H = 8
CH = 64
HC = 512
F0 = 64
NCORES = 8
NL = 2500          # real nodes per core
NLP = 2560         # padded nodes per core (20 groups of 128)
G = 20             # dst groups (= node tiles) per core
NT = 20
NEG_SLOPE = 0.2

_CACHE = {}


# ---------------------------------------------------------------- host prep

def _wrap16(idx, reps=8):
    """Wrap indices into the [16, n/16] layout and replicate across `reps`
    groups of 16 partitions (dma_gather / ap_gather index convention)."""
    idx = np.asarray(idx, np.int16)
    n = idx.shape[0]
    assert n % 16 == 0
    w = np.zeros((16, n // 16), np.int16)
    w[np.arange(n) % 16, np.arange(n) // 16] = idx
    return np.tile(w, (reps, 1))


def _host_prep(inputs):
    f32 = lambda k: np.asarray(inputs[k], np.float32)
    ei = np.asarray(inputs["edge_index"]).astype(np.int64)
    batch = np.asarray(inputs["batch_index"]).astype(np.int64)
    loop = np.arange(N, dtype=np.int64)
    src = np.concatenate([ei[0], loop])
    dst = np.concatenate([ei[1], loop])
    order = np.argsort(dst, kind="stable")
    src_s = src[order]
    dst_s = dst[order]

    core_of = dst_s // NL
    per_core = []
    maxg = 0
    for c in range(NCORES):
        m = core_of == c
        sc = src_s[m]
        dl = dst_s[m] - c * NL
        gid = dl // 128
        cnt = np.bincount(gid, minlength=G)
        maxg = max(maxg, int(cnt.max()))
        per_core.append((sc, dl, gid, cnt))
    TG = (maxg + 127) // 128          # tiles per dst group
    TG = ((TG + 3) // 4) * 4          # multiple of 4 (supertile = gather unit)
    T = G * TG                        # edge tiles per core per layer
    EP = T * 128

    srcw_l, dstw_l, mt_l = [], [], []
    for c in range(NCORES):
        sc, dl, gid, cnt = per_core[c]
        src_pad = np.zeros(EP, np.int64)
        dst_pad = np.zeros(EP, np.int64)
        valid = np.zeros(EP, bool)
        off_in = np.concatenate([[0], np.cumsum(cnt)])
        for g in range(G):
            o = g * TG * 128
            k = int(cnt[g])
            src_pad[o:o + k] = sc[off_in[g]:off_in[g] + k]
            dst_pad[o:o + k] = dl[off_in[g]:off_in[g] + k]
            valid[o:o + k] = True
        src_gidx = (src_pad // NL) * NLP + (src_pad % NL)
        mt = np.zeros((T, 128, 128), np.float32)
        epos = np.arange(EP)
        v = valid
        mt[epos[v] // 128, epos[v] % 128,
           (dst_pad[v] - (epos[v] // (TG * 128)) * 128)] = 1
        srcw_l.append(_wrap16(src_gidx))
        dstw_l.append(_wrap16(dst_pad))
        mt_l.append(mt)

    counts = np.bincount(batch, minlength=B)
    nonempty = (counts > 0).astype(np.float32)
    batchm_l, scanmask_l, lastidx_l, pres_l = [], [], [], []
    for c in range(NCORES):
        bl = batch[c * NL:(c + 1) * NL]
        bm = np.zeros((NT, 128, B), np.float32)
        rows = np.arange(NL)
        bm[rows // 128, rows % 128, bl] = 1
        batchm_l.append(bm)
        # additive reset mask for the running-max scan:
        # 0 inside a graph's run, -1e38 at run starts (and padding)
        sm = np.full(NLP, -1e38, np.float32)
        sm[1:NL] = np.where(bl[1:] == bl[:-1], 0.0, -1e38).astype(np.float32)
        scanmask_l.append(np.tile(sm[None, :], (64, 1)))
        pres = np.zeros(B, np.float32)
        pres[np.unique(bl)] = 1.0
        idx_sorted = np.searchsorted(bl, np.arange(B), side="right") - 1
        last = np.clip(idx_sorted, 0, NL - 1)
        lastidx_l.append(_wrap16(last, reps=4))
        pres_l.append(np.tile(pres[None, :], (64, 1)).astype(np.float32))

    meta = dict(TG=TG, T=T, EP=EP)

    shared = dict(
        wl0=f32("Wl0"), wr0=f32("Wr0"),
        wl1=f32("Wl1"), wr1=f32("Wr1"),
        wl2=f32("Wl2"), wr2=f32("Wr2"),
        attrep=np.stack([np.tile(f32("att0").reshape(1, HC), (128, 1)),
                         np.tile(f32("att1").reshape(1, HC), (128, 1)),
                         np.tile(f32("att2").reshape(1, HC), (128, 1))]),
        brep=np.stack([np.tile(f32("b0")[None, :], (128, 1)),
                       np.tile(f32("b1")[None, :], (128, 1))]),
        b2rep=np.tile(f32("b2")[None, :], (128, 1)),
        id128f=np.eye(128, dtype=np.float32),
        wawrep=np.tile(f32("w_aw").reshape(1, CH), (128, 1)),
        bawcol=np.full((128, 1), float(f32("b_aw")[0]), np.float32),
        wm1=f32("Wm1"),
        bm1col=f32("bm1").reshape(128, 1),
        aprelucol=np.full((128, 1), float(f32("a_prelu")[0]), np.float32),
        wm2=f32("Wm2"),
        bm2col=f32("bm2").reshape(1, 1),
        nonemptyrep=np.tile(nonempty[None, :], (64, 1)).astype(np.float32),
    )

    x = f32("x")
    in_maps = []
    for c in range(NCORES):
        xT0 = np.zeros((F0, NLP), np.float32)
        xT0[:, :NL] = x[c * NL:(c + 1) * NL].T
        m = dict(shared)
        m.update(
            xT0=xT0,
            srcw=srcw_l[c], dstw=dstw_l[c],
            mt=mt_l[c],
            batchm=batchm_l[c],
            scanmask=scanmask_l[c],
            lastidx=lastidx_l[c],
            presrep=pres_l[c],
        )
        in_maps.append(m)
    return meta, in_maps


# ---------------------------------------------------------------- device build

def _build(TG, debug=False):
    import concourse.bacc as bacc
    import concourse.mybir as mybir
    import concourse.tile as tile

    F32 = mybir.dt.float32
    I16 = mybir.dt.int16
    AF = mybir.ActivationFunctionType
    ALU = mybir.AluOpType
    AX = mybir.AxisListType

    T = G * TG
    NSUP = TG // 4            # 4-tile supertiles per dst group
    RG = [list(range(NCORES))]

    nc = bacc.Bacc("TRN2", target_bir_lowering=False, debug=False,
                   num_devices=NCORES)

    # ---- I/O ----
    DT = nc.dram_tensor
    xT0 = DT("xT0", [F0, NLP], F32, kind="ExternalInput")
    wl0 = DT("wl0", [F0, HC], F32, kind="ExternalInput")
    wr0 = DT("wr0", [F0, HC], F32, kind="ExternalInput")
    wl1 = DT("wl1", [HC, HC], F32, kind="ExternalInput")
    wr1 = DT("wr1", [HC, HC], F32, kind="ExternalInput")
    wl2 = DT("wl2", [HC, HC], F32, kind="ExternalInput")
    wr2 = DT("wr2", [HC, HC], F32, kind="ExternalInput")
    attrep = DT("attrep", [3, 128, HC], F32, kind="ExternalInput")
    brep = DT("brep", [2, 128, HC], F32, kind="ExternalInput")
    b2rep = DT("b2rep", [128, CH], F32, kind="ExternalInput")
    id128f = DT("id128f", [128, 128], F32, kind="ExternalInput")
    srcw = DT("srcw", [128, T * 8], I16, kind="ExternalInput")
    dstw = DT("dstw", [128, T * 8], I16, kind="ExternalInput")
    mt = DT("mt", [T, 128, 128], F32, kind="ExternalInput")
    batchm = DT("batchm", [NT, 128, B], F32, kind="ExternalInput")
    scanmask = DT("scanmask", [64, NLP], F32, kind="ExternalInput")
    lastidx = DT("lastidx", [64, B // 16], I16, kind="ExternalInput")
    presrep = DT("presrep", [64, B], F32, kind="ExternalInput")
    nonemptyrep = DT("nonemptyrep", [64, B], F32, kind="ExternalInput")
    wawrep = DT("wawrep", [128, CH], F32, kind="ExternalInput")
    bawcol = DT("bawcol", [128, 1], F32, kind="ExternalInput")
    wm1 = DT("wm1", [128, 128], F32, kind="ExternalInput")
    bm1col = DT("bm1col", [128, 1], F32, kind="ExternalInput")
    aprelucol = DT("aprelucol", [128, 1], F32, kind="ExternalInput")
    wm2 = DT("wm2", [128, 1], F32, kind="ExternalInput")
    bm2col = DT("bm2col", [1, 1], F32, kind="ExternalInput")
    outT = DT("outT", [1, B], F32, kind="ExternalOutput")
    if debug:
        dbg_xl0 = DT("dbg_xl0", [NCORES * NLP, HC], F32, kind="ExternalOutput")
        dbg_hT = DT("dbg_hT", [3, 4, 128, NLP], F32, kind="ExternalOutput")
        dbg_h2T = DT("dbg_h2T", [64, NLP], F32, kind="ExternalOutput")

    # internal DRAM
    xl_sh = DT("xl_sh", [NLP, HC], F32, kind="Internal")
    xr_sh = DT("xr_sh", [NLP, HC], F32, kind="Internal")
    xl_full = DT("xl_full", [NCORES * NLP, HC], F32, kind="Internal",
                 addr_space="Shared")
    hT_dram = DT("hT_dram", [4, 128, NLP], F32, kind="Internal")
    pmax_in = DT("pmax_in", [64, B], F32, kind="Internal")
    pmax_out = DT("pmax_out", [64, B], F32, kind="Internal", addr_space="Shared")
    psum_in = DT("psum_in", [64, B], F32, kind="Internal")
    psum_out = DT("psum_out", [64, B], F32, kind="Internal", addr_space="Shared")

    wl_l = [wl0, wl1, wl2]
    wr_l = [wr0, wr1, wr2]

    with tile.TileContext(nc) as tc:
        with tc.tile_pool(name="res", bufs=1) as res, \
             tc.tile_pool(name="wts", bufs=1) as wts, \
             tc.tile_pool(name="gat", bufs=3) as gat, \
             tc.tile_pool(name="mtp", bufs=2) as mtp, \
             tc.tile_pool(name="edg", bufs=2) as edg, \
             tc.tile_pool(name="sml", bufs=2) as sml, \
             tc.tile_pool(name="nod", bufs=2) as nod, \
             tc.tile_pool(name="psU", bufs=2, space="PSUM") as psUp, \
             tc.tile_pool(name="psS", bufs=2, space="PSUM") as psSp, \
             tc.tile_pool(name="pax", bufs=3, space="PSUM") as pax:

            # ---- resident SBUF ----
            idf_sb = res.tile([128, 128], F32)
            nc.sync.dma_start(idf_sb[:], id128f[:])
            srcw_sb = res.tile([128, T * 8], I16)
            nc.sync.dma_start(srcw_sb[:], srcw[:])
            dstw_sb = res.tile([128, T * 8], I16)
            nc.sync.dma_start(dstw_sb[:], dstw[:])
            h2T = res.tile([64, NLP], F32)
            scanmask_sb = res.tile([64, NLP], F32)
            nc.sync.dma_start(scanmask_sb[:], scanmask[:])
            lastidx_sb = res.tile([64, B // 16], I16)
            nc.sync.dma_start(lastidx_sb[:], lastidx[:])
            presrep_sb = res.tile([64, B], F32)
            nc.sync.dma_start(presrep_sb[:], presrep[:])
            nonempty_sb = res.tile([64, B], F32)
            nc.sync.dma_start(nonempty_sb[:], nonemptyrep[:])
            b2pack = res.tile([128, 2 * CH], F32)     # b2rep | wawrep
            nc.sync.dma_start(b2pack[:, 0:CH], b2rep[:])
            nc.sync.dma_start(b2pack[:, CH:2 * CH], wawrep[:])
            colpack = res.tile([128, 4], F32)  # baw | bm1 | aprelu
            nc.sync.dma_start(colpack[:, 0:1], bawcol[:])
            nc.sync.dma_start(colpack[:, 1:2], bm1col[:])
            nc.sync.dma_start(colpack[:, 2:3], aprelucol[:])
            bm2_sb = res.tile([1, 1], F32)
            nc.sync.dma_start(bm2_sb[:], bm2col[:])
            wm1_sb = res.tile([128, 128], F32)
            nc.sync.dma_start(wm1_sb[:], wm1[:])
            wm2_sb = res.tile([128, 1], F32)
            nc.sync.dma_start(wm2_sb[:], wm2[:])
            psumT = nc.alloc_psum_tensor("psumT", [64, B], F32)

            for l in range(3):
                chunks = 1 if l == 0 else 4
                KP = F0 if l == 0 else 128
                wlt = wts.tile([128, 4, HC], F32, tag="wlt")
                wrt = wts.tile([128, 4, HC], F32, tag="wrt")
                for k in range(chunks):
                    nc.sync.dma_start(wlt[:KP, k, :], wl_l[l][k * 128:k * 128 + KP, :])
                    nc.sync.dma_start(wrt[:KP, k, :], wr_l[l][k * 128:k * 128 + KP, :])
                att_sb = wts.tile([128, HC], F32, tag="att")
                nc.sync.dma_start(att_sb[:], attrep[l])
                if l < 2:
                    brep_sb = wts.tile([128, HC], F32, tag="brep")
                    nc.sync.dma_start(brep_sb[:], brep[l])

                # ---- node-level matmuls -> xl_sh / xr_sh ----
                for j in range(NT):
                    psA = pax.tile([128, HC], F32, tag="pax")
                    psB = pax.tile([128, HC], F32, tag="pax")
                    for k in range(chunks):
                        lt = nod.tile([128, 128], F32, tag="lt")
                        if l == 0:
                            nc.sync.dma_start(
                                lt[:KP, :], xT0.ap()[:, j * 128:(j + 1) * 128])
                        else:
                            nc.sync.dma_start(
                                lt[:], hT_dram.ap()[k, :, j * 128:(j + 1) * 128])
                        nc.tensor.matmul(psA[:], lt[:KP, :], wlt[:KP, k, :],
                                         start=(k == 0), stop=(k == chunks - 1))
                        nc.tensor.matmul(psB[:], lt[:KP, :], wrt[:KP, k, :],
                                         start=(k == 0), stop=(k == chunks - 1))
                    xlt = nod.tile([128, HC], F32, tag="xlt")
                    nc.scalar.copy(xlt[:], psA[:])
                    nc.sync.dma_start(xl_sh[j * 128:(j + 1) * 128, :], xlt[:])
                    xrt = nod.tile([128, HC], F32, tag="xrt")
                    nc.scalar.copy(xrt[:], psB[:])
                    nc.sync.dma_start(xr_sh[j * 128:(j + 1) * 128, :], xrt[:])

                # ---- all-gather xl ----
                nc.gpsimd.collective_compute(
                    "AllGather", ALU.bypass, replica_groups=RG,
                    ins=[xl_sh.ap()], outs=[xl_full.ap()])
                if debug and l == 0:
                    nc.sync.dma_start(dbg_xl0.ap(), xl_full.ap())

                # ---- edge pipeline ----
                for g in range(G):
                    t0g = g * TG
                    mtg = mtp.tile([128, TG, 128], F32, tag="mtg")
                    nc.sync.dma_start(
                        mtg[:], mt.ap()[t0g:t0g + TG].rearrange("t p d -> p t d"))
                    psU = psUp.tile([128, HC], F32, tag="psU")
                    psS = psSp.tile([128, HC], F32, tag="psS")
                    for s in range(NSUP):
                        t0 = t0g + s * 4
                        xlg = gat.tile([128, 4, HC], F32, tag="xlg")
                        nc.gpsimd.dma_gather(
                            xlg[:], xl_full.ap(), srcw_sb[:, t0 * 8:(t0 + 4) * 8],
                            512, 512, HC)
                        xrg = gat.tile([128, 4, HC], F32, tag="xrg")
                        nc.gpsimd.dma_gather(
                            xrg[:], xr_sh.ap(), dstw_sb[:, t0 * 8:(t0 + 4) * 8],
                            512, 512, HC)
                        Et = edg.tile([128, 4, HC], F32, tag="Et")
                        nc.vector.tensor_add(Et[:], xlg[:], xrg[:])
                        Rt = edg.tile([128, 4, HC], F32, tag="Rt")
                        # leaky_relu(x) = max(0.2*x, x) in one DVE op
                        nc.vector.scalar_tensor_tensor(
                            Rt[:], Et[:], NEG_SLOPE, Et[:], ALU.mult, ALU.max)
                        # RA = R * att  (overwrites Et; E is dead)
                        nc.vector.tensor_mul(
                            Et[:], Rt[:],
                            att_sb[:].unsqueeze(1).broadcast_to([128, 4, HC]))
                        lex = sml.tile([128, 4, 2 * H], F32, tag="lex")
                        nc.vector.reduce_sum(
                            lex[:, :, 0:H],
                            Et[:].rearrange("p t (h c) -> p t h c", h=H),
                            axis=AX.X)
                        nc.scalar.activation(lex[:, :, H:2 * H], lex[:, :, 0:H],
                                             AF.Exp)
                        # Y = xl * exp(logits)  (overwrites Rt; R is dead)
                        nc.vector.tensor_mul(
                            Rt[:].rearrange("p t (h c) -> p t h c", h=H),
                            xlg[:].rearrange("p t (h c) -> p t h c", h=H),
                            lex[:, :, H:2 * H].unsqueeze(3).broadcast_to(
                                [128, 4, H, CH]))
                        for tt in range(4):
                            t = t0 + tt
                            nc.tensor.matmul(psU[:], mtg[:, t - t0g, :],
                                             Rt[:, tt, :],
                                             start=(t == t0g),
                                             stop=(t == t0g + TG - 1))
                            nc.tensor.matmul(psS[:, :H], mtg[:, t - t0g, :],
                                             lex[:, tt, H:2 * H],
                                             start=(t == t0g),
                                             stop=(t == t0g + TG - 1))
                    # ---- group epilogue ----
                    rp = sml.tile([128, 3 * H], F32, tag="rp")  # srec|rec|rec8
                    nc.vector.tensor_scalar_add(rp[:, 0:H], psS[:, :H], 1e-20)
                    nc.vector.reciprocal(rp[:, H:2 * H], rp[:, 0:H])
                    gsl = slice(g * 128, (g + 1) * 128)
                    if l < 2:
                        h1 = nod.tile([128, HC], F32, tag="h1")
                        nc.vector.tensor_mul(
                            h1[:].rearrange("p (h c) -> p h c", h=H),
                            psU[:].rearrange("p (h c) -> p h c", h=H),
                            rp[:, H:2 * H].unsqueeze(2).broadcast_to(
                                [128, H, CH]))
                        nc.vector.tensor_add(h1[:], h1[:], brep_sb[:])
                        ptr = pax.tile([128, HC], F32, tag="pax")
                        for k in range(4):
                            nc.tensor.transpose(ptr[:, k * 128:(k + 1) * 128],
                                                h1[:, k * 128:(k + 1) * 128],
                                                idf_sb[:])
                        hsl = nod.tile([128, HC], F32, tag="hsl")
                        nc.scalar.copy(hsl[:], ptr[:])
                        for k in range(4):
                            nc.sync.dma_start(hT_dram.ap()[k, :, gsl],
                                              hsl[:, k * 128:(k + 1) * 128])
                    else:
                        nc.vector.tensor_scalar_mul(rp[:, 2 * H:3 * H],
                                                    rp[:, H:2 * H], 1.0 / H)
                        v32 = nod.tile([128, HC], F32, tag="h1")
                        nc.vector.tensor_mul(
                            v32[:].rearrange("p (h c) -> p h c", h=H),
                            psU[:].rearrange("p (h c) -> p h c", h=H),
                            rp[:, 2 * H:3 * H].unsqueeze(2).broadcast_to(
                                [128, H, CH]))
                        hp = nod.tile([128, 3 * CH], F32, tag="hp")  # h2|h2b|wa
                        nc.vector.reduce_sum(
                            hp[:, 0:CH],
                            v32[:].rearrange("p (h c) -> p c h", h=H),
                            axis=AX.X)
                        nc.vector.tensor_add(hp[:, CH:2 * CH], hp[:, 0:CH],
                                             b2pack[:, 0:CH])
                        nc.vector.tensor_mul(hp[:, 2 * CH:3 * CH],
                                             hp[:, CH:2 * CH],
                                             b2pack[:, CH:2 * CH])
                        ws = sml.tile([128, 2], F32, tag="ws")
                        nc.vector.reduce_sum(ws[:, 0:1], hp[:, 2 * CH:3 * CH],
                                             axis=AX.X)
                        nc.scalar.activation(ws[:, 1:2], ws[:, 0:1], AF.Sigmoid,
                                             bias=colpack[:, 0:1])
                        wh2 = nod.tile([128, CH], F32, tag="wh2")
                        nc.vector.tensor_scalar(wh2[:], hp[:, CH:2 * CH],
                                                ws[:, 1:2], None, ALU.mult)
                        bmg = mtp.tile([128, B], F32, tag="bmg")
                        nc.sync.dma_start(bmg[:], batchm.ap()[g])
                        nc.tensor.matmul(psumT[:], wh2[:], bmg[:],
                                         start=(g == 0), stop=(g == G - 1))
                        ptr2 = pax.tile([128, HC], F32, tag="pax")
                        nc.tensor.transpose(ptr2[:64, :128], hp[:, CH:2 * CH],
                                            idf_sb[:])
                        nc.scalar.copy(h2T[:, gsl], ptr2[:64, :128])
                if debug and l < 2:
                    for k in range(4):
                        nc.sync.dma_start(dbg_hT.ap()[l, k], hT_dram.ap()[k])
            if debug:
                nc.sync.dma_start(dbg_h2T.ap(), h2T[:])

            # ---- pooling tail ----
            scano = res.tile([64, NLP], F32)
            nc.vector.tensor_tensor_scan(
                scano[:], scanmask_sb[:], h2T[:], 0.0, ALU.add, ALU.max)
            pool6 = res.tile([64, 6, B], F32)
            nc.gpsimd.ap_gather(
                pool6[:, 0, :].unsqueeze(2), scano[:].unsqueeze(2),
                lastidx_sb[:], 64, NLP, 1, B)
            nc.vector.tensor_scalar(pool6[:, 1, :], presrep_sb[:], 1e30, -1e30,
                                    ALU.mult, ALU.add)
            nc.vector.tensor_mul(pool6[:, 2, :], pool6[:, 0, :], presrep_sb[:])
            nc.vector.tensor_add(pool6[:, 3, :], pool6[:, 2, :], pool6[:, 1, :])
            nc.sync.dma_start(pmax_in[:], pool6[:, 3, :])
            nc.gpsimd.collective_compute(
                "AllReduce", ALU.max, replica_groups=RG,
                ins=[pmax_in.ap()], outs=[pmax_out.ap()])
            nc.sync.dma_start(pool6[:, 4, :], pmax_out[:])
            pscp = res.tile([64, B], F32)
            nc.scalar.copy(pscp[:], psumT[:])
            nc.sync.dma_start(psum_in[:], pscp[:])
            nc.gpsimd.collective_compute(
                "AllReduce", ALU.add, replica_groups=RG,
                ins=[psum_in.ap()], outs=[psum_out.ap()])

            gT = res.tile([128, B], F32)
            nc.vector.tensor_mul(gT[0:64, :], pool6[:, 4, :], nonempty_sb[:])
            # partition-shifting copy (rows 0..63 -> 64..127) must be a DMA
            nc.gpsimd.dma_start(gT[64:128, :], psum_out[:])
            psz = pax.tile([128, B], F32, tag="pax")
            nc.tensor.matmul(psz[:], wm1_sb[:], gT[:])
            zb = res.tile([128, B], F32)
            nc.scalar.activation(zb[:], psz[:], AF.Identity,
                                 bias=colpack[:, 1:2])
            z1 = res.tile([128, B], F32)
            # prelu(z) = max(a*z, z) for 0 <= a <= 1
            nc.vector.scalar_tensor_tensor(
                z1[:], zb[:], colpack[:, 2:3], zb[:], ALU.mult, ALU.max)
            pso = pax.tile([128, B], F32, tag="pax")
            nc.tensor.matmul(pso[:1, :], wm2_sb[:], z1[:])
            osb = res.tile([1, B], F32)
            nc.scalar.activation(osb[:], pso[:1, :], AF.Identity,
                                 bias=bm2_sb[:, 0:1])
            nc.sync.dma_start(outT[:], osb[:])

    nc.compile()
    return nc


# ---------------------------------------------------------------- entry point

def kernel(**inputs):
    from concourse.bass_utils import run_bass_kernel_spmd

    meta, in_maps = _host_prep(inputs)
    TG = meta["TG"]
    if TG not in _CACHE:
        _CACHE[TG] = _build(TG)
    nc = _CACHE[TG]
    res = run_bass_kernel_spmd(nc, in_maps, core_ids=list(range(NCORES)))
    out = np.asarray(res.results[0]["outT"], np.float32).reshape(B, 1)
    return out
